# revision 28
# baseline (speedup 1.0000x reference)
"""AttentionBlock (GroupNorm + single-head-per-core spatial attention + proj)
for Trainium2, 8 NeuronCores.

Sharding: core i handles (batch b = i//4, head h = i%4).  Each core computes
its head's attention output projected through its slice of proj_w's input
channels; the host sums the 4 per-head partials per batch (tensor-parallel
unshard) and adds residual + biases.

Reference semantics (B=2, C=128, H=W=64, heads=4, groups=32, eps=1e-5):
  h   = groupnorm(x) * nw + nb
  qkv = qkv_w @ h + qkv_b          (1x1 conv == channel matmul)
  S   = (q^T k) / sqrt(32); A = softmax(S, axis=t); out = v A^T
  y   = proj_w @ out + proj_b + x

v2 structure (vs. the earlier single-engine-exp version):
  - exp split across ScalarE (exact table exp) and VectorE (Schraudolph
    bit-trick exp: int32(A*S + B) bitcast to float ~= e^S, ~3% per-weight
    err that washes out in the softmax-weighted sums; validated 1.6e-3
    scale-rel worst case with ALL quads approximated).
  - s-chunks processed in PAIRS: each [128,1024] S^T tile = one t-block x
    two s-chunks, halving per-element weight loads; the two AV accumulator
    slabs live at PSUM partitions 0:33 / 64:97 of one bank and run as
    column-tiled concurrent matmuls (tile_position cols 0 / 64).
  - k bias dropped entirely (constant-in-t terms cancel in softmax over t);
    groupnorm folded into q/k/v weights; f32r matmuls via bitcast (no
    separate f32r copy of X).
  - PE warm-up: dummy matmuls during the DMA/stats preamble trip the HAM
    activity monitor so the attention stream runs at 2.4 GHz from the start.
  - softmax denominators ride row 0 of each AV slab (ones column in v^T);
    per-pair both reciprocals computed in one strided 2-partition DVE op.
  - residual, proj_b, and the v-bias term (proj_w @ bv) are added on host.
"""

import sys

sys.path.insert(0, "/opt/trn_rl_repo")

import numpy as np

import concourse.bass as bass
import concourse.tile as tile
from concourse import bacc, mybir
from concourse.bass_utils import run_bass_kernel_spmd

F32 = mybir.dt.float32
F32R = mybir.dt.float32r
I16 = mybir.dt.int16
BF16 = mybir.dt.bfloat16

B, C, HW = 2, 128, 4096
NH, DH = 4, 32
NG, GS = 32, 4  # groups, channels per group
EPS = 1e-5
SCALE = 1.0 / np.sqrt(DH)

# Schraudolph exp in bf16: int16((A*x + B) / 2^16) bit-pattern == bf16(e^x)
# (top 16 bits of the classic int32 f32-bit-pattern trick; ~3% sawtooth err)
A_EXP = float((1 << 23) / np.log(2.0) * SCALE / 65536.0)  # scale folded in
B_EXP = float((127.0 * (1 << 23) - 0.0437 * (1 << 23)) / 65536.0)

N_CORES = 8
N_WARM = 36  # HAM warm-up matmuls during preamble
LAG = 4  # units between exp(u) and AV(u)
# 14 of 32 units per pair go to the DVE Schraudolph path
DVE_UNITS = frozenset(u for u in range(32) if (u % 16) in (1, 3, 5, 7, 9, 11, 13))

MUL = mybir.AluOpType.mult
ADD = mybir.AluOpType.add
SUB = mybir.AluOpType.subtract


def build_program():
    nc = bacc.Bacc("TRN2", target_bir_lowering=False, debug=False)

    def din(name, shape):
        return nc.dram_tensor(name, shape, F32, kind="ExternalInput").ap()

    xs = din("xs", [C, HW])
    cpk = din("cpk", [C, 323])  # packed: wqT4|wkT4|wvT|bq4|gs|nw|nb
    pwTa = din("pwTa", [DH, C])
    g2 = din("g2", [NG, C])
    out_d = nc.dram_tensor("out_p", [C, HW], F32, kind="ExternalOutput").ap()
    den_d = nc.dram_tensor("den_p", [1, HW], F32, kind="ExternalOutput").ap()

    ident = mybir.ActivationFunctionType.Identity
    fcopy = mybir.ActivationFunctionType.Copy
    fexp = mybir.ActivationFunctionType.Exp
    fsqrt = mybir.ActivationFunctionType.Sqrt
    fln = mybir.ActivationFunctionType.Ln

    with tile.TileContext(nc) as tc:
        with (
            tc.tile_pool(name="consts", bufs=1) as consts,
            tc.tile_pool(name="xpool", bufs=1) as xpool,
            tc.tile_pool(name="qk", bufs=1) as qkpool,
            tc.tile_pool(name="vt", bufs=1) as vtpool,
            tc.tile_pool(name="small", bufs=8) as small,
            tc.tile_pool(name="epool", bufs=5) as epool,
            tc.tile_pool(name="epi", bufs=2) as epi,
        ):
            # ---------------- input DMAs ----------------
            cbuf = consts.tile([C, 323], F32)
            nc.scalar.dma_start(cbuf[:], cpk[:])
            c_wqT4 = cbuf[:, 0:128]
            c_wkT4 = cbuf[:, 128:256]
            c_wvT = cbuf[:, 256:288]
            c_bq4 = cbuf[:, 288:289]
            c_gs = cbuf[:, 289:321]
            c_nw = cbuf[:, 321:322]
            c_nb = cbuf[:, 322:323]
            # proj lhsT, twice (partitions 0:33 and 64:97): row 0/64 = runtime
            # addvec, rows 1:33 / 65:97 = pwTa
            pwaug2 = consts.tile([97, C], F32R)
            c_pwTa = consts.tile([DH, C], F32)
            nc.scalar.dma_start(c_pwTa[:], pwTa[:])
            c_g2 = consts.tile([NG, C], F32)
            nc.scalar.dma_start(c_g2[:], g2[:])
            X = xpool.tile([C, HW], F32)
            dma_engs = [nc.sync, nc.scalar, nc.gpsimd, nc.sync]
            for j in range(4):
                dma_engs[j].dma_start(
                    X[:, 1024 * j : 1024 * (j + 1)], xs[:, 1024 * j : 1024 * (j + 1)]
                )
            # f32r copy of X (verifier requires a rounding producer for f32r
            # matmul inputs); split ACT/DVE, pipelined behind the chunk DMAs
            Xr_t = xpool.tile([C, HW], F32R, tag="Xr")
            for j in range(4):
                if j % 2 == 0:
                    nc.scalar.copy(
                        out=Xr_t[:, 1024 * j : 1024 * (j + 1)],
                        in_=X[:, 1024 * j : 1024 * (j + 1)],
                    )
                else:
                    nc.vector.tensor_copy(
                        out=Xr_t[:, 1024 * j : 1024 * (j + 1)],
                        in_=X[:, 1024 * j : 1024 * (j + 1)],
                    )
            Xr = Xr_t[:]

            # small consts
            eps_t = consts.tile([NG, 1], F32)
            nc.vector.memset(eps_t[:], EPS)
            ones_f = consts.tile([128, 1], F32)
            nc.vector.memset(ones_f[:], 1.0)
            warm_0 = consts.tile([128, 512], F32)
            nc.gpsimd.memset(warm_0[:], 0.0)
            warm_w = consts.tile([128, 128], F32R)
            nc.vector.tensor_copy(out=warm_w[:], in_=warm_0[:, 0:128])
            warm_r = consts.tile([128, 512], F32R)
            nc.vector.tensor_copy(out=warm_r[:], in_=warm_0[:])
            tblw = small.tile([NG, 1], F32)

            # per-t-block AV lhsT: cols 0:32 = v, col 32 = ones (denom row)
            v_t = vtpool.tile([128, 32, DH + 1], BF16)
            nc.vector.tensor_copy(
                out=v_t[:, :, DH], in_=ones_f[:, 0:1].to_broadcast([128, 32])
            )
            q_sb = qkpool.tile([128, HW], F32R, tag="q")
            k_sb = qkpool.tile([128, HW], F32R, tag="k")

            with (
                tc.tile_pool(name="warmp", bufs=1, space="PSUM") as warmp,
                tc.tile_pool(name="pp", bufs=1, space="PSUM") as pp,
                tc.tile_pool(name="buildp", bufs=2, space="PSUM") as buildp,
            ):
                # ACT sqrt-table load early (hides under DMA); the Exp set is
                # loaded right after the real sqrt below.
                nc.scalar.activation(out=tblw[:], in_=eps_t[:], func=fsqrt, scale=1.0)

                # HAM warm-up: keep the PE busy through the preamble so the
                # attention stream starts at 2.4 GHz.
                wps = warmp.tile([128, 512], F32)
                for i in range(N_WARM):
                    nc.tensor.matmul(
                        wps[:],
                        lhsT=warm_w[:],
                        rhs=warm_r[:],
                        start=(i == 0),
                        stop=(i == N_WARM - 1),
                    )

                # ---------------- groupnorm stats ----------------
                Xg = X[:].rearrange("c (n f) -> c n f", f=512)
                stats = small.tile([C, 8, 6], F32)
                for i in range(8):
                    nc.vector.bn_stats(out=stats[:, i, :], in_=Xg[:, i, :])
                mv = small.tile([C, 2], F32)
                nc.vector.bn_aggr(out=mv[:], in_=stats[:])
                # mv2 = [mean_c, E[x^2]_c]
                mv2 = small.tile([C, 2], F32)
                nc.vector.tensor_copy(out=mv2[:, 0:1], in_=mv[:, 0:1])
                nc.vector.tensor_tensor(
                    out=mv2[:, 1:2], in0=mv[:, 0:1], in1=mv[:, 0:1], op=MUL
                )
                nc.vector.tensor_tensor(
                    out=mv2[:, 1:2], in0=mv2[:, 1:2], in1=mv[:, 1:2], op=ADD
                )
                gstat_ps = pp.tile([NG, 2], F32, tag="pp")
                nc.tensor.matmul(gstat_ps[:], lhsT=c_gs[:], rhs=mv2[:])
                gstat = small.tile([NG, 2], F32)
                nc.vector.tensor_copy(out=gstat[:], in_=gstat_ps[:])
                varg = small.tile([NG, 1], F32)
                nc.vector.tensor_tensor(
                    out=varg[:], in0=gstat[:, 0:1], in1=gstat[:, 0:1], op=MUL
                )
                nc.vector.tensor_tensor(
                    out=varg[:], in0=gstat[:, 1:2], in1=varg[:], op=SUB
                )
                stdg = small.tile([NG, 1], F32)
                nc.scalar.activation(
                    out=stdg[:], in_=varg[:], func=fsqrt, bias=eps_t[:], scale=1.0
                )
                # switch ACT to the exp table set now (load hides under builds)
                nc.scalar.activation(out=tblw[:], in_=eps_t[:], func=fexp, scale=1.0)
                rstdg = small.tile([NG, 1], F32)
                nc.vector.reciprocal(out=rstdg[:], in_=stdg[:])
                gexp = small.tile([NG, 2], F32)
                nc.vector.tensor_copy(out=gexp[:, 0:1], in_=gstat[:, 0:1])
                nc.vector.tensor_copy(out=gexp[:, 1:2], in_=rstdg[:])
                mrc_ps = pp.tile([C, 2], F32, tag="pp")
                nc.tensor.matmul(mrc_ps[:], lhsT=c_g2[:], rhs=gexp[:])
                mrc = small.tile([C, 2], F32)
                nc.vector.tensor_copy(out=mrc[:], in_=mrc_ps[:])
                # scale_c = rstd_c * nw ; shift_c = nb - mean_c*scale_c
                scale_c = small.tile([C, 1], F32)
                nc.vector.tensor_tensor(
                    out=scale_c[:], in0=mrc[:, 1:2], in1=c_nw[:], op=MUL
                )
                shift_c = small.tile([C, 1], F32)
                nc.vector.tensor_tensor(
                    out=shift_c[:], in0=mrc[:, 0:1], in1=scale_c[:], op=MUL
                )
                nc.vector.tensor_tensor(
                    out=shift_c[:], in0=c_nb[:], in1=shift_c[:], op=SUB
                )
                # folded weights
                wq_f = consts.tile([C, 128], F32R)
                nc.vector.tensor_scalar_mul(out=wq_f[:], in0=c_wqT4[:], scalar1=scale_c[:])
                wk_f = consts.tile([C, 128], F32R)
                nc.vector.tensor_scalar_mul(out=wk_f[:], in0=c_wkT4[:], scalar1=scale_c[:])
                wv_f = consts.tile([C, DH], F32R)
                nc.vector.tensor_scalar_mul(out=wv_f[:], in0=c_wvT[:], scalar1=scale_c[:])
                # adjusted q bias (4x-replicated); k needs no bias at all
                bq_ps = pp.tile([128, 1], F32, tag="pp")
                nc.tensor.matmul(bq_ps[:], lhsT=c_wqT4[:], rhs=shift_c[:])
                bq_f = small.tile([128, 1], F32)
                nc.vector.tensor_tensor(
                    out=bq_f[:], in0=bq_ps[:], in1=c_bq4[:], op=ADD
                )
                # v shift term -> proj addvec rows (partitions 0 and 64)
                vs_ps = pp.tile([DH, 1], F32, tag="pp")
                nc.tensor.matmul(vs_ps[:], lhsT=c_wvT[:], rhs=shift_c[:])
                vsum = small.tile([DH, 1], F32)
                nc.vector.tensor_copy(out=vsum[:], in_=vs_ps[:])
                av_ps = pp.tile([97, C], F32, tag="av")
                nc.tensor.matmul(av_ps[32:33, :], lhsT=vsum[:], rhs=c_pwTa[:])
                nc.tensor.matmul(
                    av_ps[96:97, :], lhsT=vsum[:], rhs=c_pwTa[:],
                    tile_position=(0, 96),
                )
                nc.vector.tensor_copy(out=pwaug2[32:33, :], in_=av_ps[32:33, :])
                nc.vector.tensor_copy(out=pwaug2[96:97, :], in_=av_ps[96:97, :])
                # pwTa rows into both slabs (f32 -> f32r rounding producers)
                nc.vector.tensor_copy(out=pwaug2[0:32, :], in_=c_pwTa[:])
                nc.vector.tensor_copy(out=pwaug2[64:96, :], in_=c_pwTa[:])

                # ---------------- q/k builds ----------------
                for j in range(4):
                    o = 1024 * j
                    qp = buildp.tile([128, 1024], F32, tag="b")
                    nc.tensor.matmul(qp[:, 0:512], lhsT=wq_f[:], rhs=Xr[:, o : o + 512])
                    nc.tensor.matmul(
                        qp[:, 512:1024], lhsT=wq_f[:], rhs=Xr[:, o + 512 : o + 1024]
                    )
                    nc.scalar.activation(
                        out=q_sb[:, o : o + 1024], in_=qp[:], func=ident,
                        bias=bq_f[:], scale=1.0,
                    )
                    kp = buildp.tile([128, 1024], F32, tag="b")
                    nc.tensor.matmul(kp[:, 0:512], lhsT=wk_f[:], rhs=Xr[:, o : o + 512])
                    nc.tensor.matmul(
                        kp[:, 512:1024], lhsT=wk_f[:], rhs=Xr[:, o + 512 : o + 1024]
                    )
                    nc.vector.tensor_copy(out=k_sb[:, o : o + 1024], in_=kp[:])

            # ---------------- attention: 4 pairs of s-chunks ----------------
            with (
                tc.tile_pool(name="sqp", bufs=2, space="PSUM") as sqp,
                tc.tile_pool(name="accp", bufs=1, space="PSUM") as accp,
                tc.tile_pool(name="miscp", bufs=2, space="PSUM") as miscp,
            ):
                for p in range(4):
                    c0o = 1024 * p
                    c1o = 1024 * p + 512
                    acc = accp.tile([128, 512], F32, tag="acc")
                    acc1 = accp.tile([128, 512], F32, tag="acc1")
                    E_tiles = [None] * 32
                    vp = None
                    for it in range(32 + LAG):
                        u = it
                        if u < 32:
                            b = u % 4
                            SQ = sqp.tile([128, 1024], F32, tag="sq")
                            lhsK = k_sb[32 * b : 32 * (b + 1), 128 * u : 128 * (u + 1)]
                            nc.tensor.matmul(
                                SQ[:, 0:512],
                                lhsT=lhsK,
                                rhs=q_sb[32 * b : 32 * (b + 1), c0o : c0o + 512],
                                tile_position=(32 * b, 0),
                            )
                            nc.tensor.matmul(
                                SQ[:, 512:1024],
                                lhsT=lhsK,
                                rhs=q_sb[32 * b : 32 * (b + 1), c1o : c1o + 512],
                                tile_position=(32 * b, 0),
                            )
                            # v^T build rides inside pair 0 (groups of 8 blocks)
                            if p == 0 and u % 8 < 4:
                                g = u // 8
                                if u % 8 == 0:
                                    vp = miscp.tile([128, 256], F32, tag="m")
                                for jj in (2 * (u % 8), 2 * (u % 8) + 1):
                                    tb = 8 * g + jj
                                    nc.tensor.matmul(
                                        vp[:, 32 * jj : 32 * (jj + 1)],
                                        lhsT=Xr[:, 128 * tb : 128 * (tb + 1)],
                                        rhs=wv_f[:],
                                    )
                                if u % 8 == 3:
                                    nc.vector.tensor_copy(
                                        out=v_t[:, 8 * g : 8 * (g + 1), 0:DH],
                                        in_=vp[:].rearrange("p (g d) -> p g d", d=32),
                                    )
                            if u in DVE_UNITS:
                                Ei = epool.tile([128, 1024], I16, tag="ed", bufs=14)
                                nc.vector.tensor_scalar(
                                    Ei[:], SQ[:], A_EXP, B_EXP, MUL, ADD
                                )
                                E_tiles[u] = Ei[:].bitcast(BF16)
                            else:
                                Ea = epool.tile([128, 1024], BF16, tag="ea", bufs=18)
                                nc.scalar.activation(
                                    out=Ea[:], in_=SQ[:], func=fexp, scale=float(SCALE)
                                )
                                E_tiles[u] = Ea[:]
                        ua = it - LAG
                        if 0 <= ua < 32:
                            E = E_tiles[ua]
                            E_tiles[ua] = None
                            # two accumulator slabs in SEPARATE banks (the
                            # start flag's has_written clear is bank-wide);
                            # col positions 0/64 let the two streams overlap
                            nc.tensor.matmul(
                                acc[0:33, :],
                                lhsT=v_t[:, ua, :],
                                rhs=E[:, 0:512],
                                tile_position=(0, 0),
                                start=(ua == 0),
                                stop=(ua == 31),
                            )
                            nc.tensor.matmul(
                                acc1[64:97, :],
                                lhsT=v_t[:, ua, :],
                                rhs=E[:, 512:1024],
                                tile_position=(0, 64),
                                start=(ua == 0),
                                stop=(ua == 31),
                            )

                    # ---------------- pair epilogue ----------------
                    U2 = epi.tile([97, 512], F32R, tag="u2")
                    nc.scalar.activation(
                        out=U2[0:33, :], in_=acc[0:33, :], func=fcopy, scale=1.0
                    )
                    nc.scalar.activation(
                        out=U2[64:97, :], in_=acc1[64:97, :], func=fcopy, scale=1.0
                    )
                    # denominators go to the host (proj is linear, so the
                    # host divides pj by denom during unsharding)
                    nc.sync.dma_start(
                        out=den_d[:, c0o : c0o + 512], in_=U2[32:33, :].bitcast(F32)
                    )
                    nc.sync.dma_start(
                        out=den_d[:, c1o : c1o + 512], in_=U2[96:97, :].bitcast(F32)
                    )
                    pj0 = miscp.tile([C, 512], F32, tag="m")
                    nc.tensor.matmul(
                        pj0[:], lhsT=pwaug2[0:33, :], rhs=U2[0:33, :]
                    )
                    osb0 = epi.tile([C, 512], F32, tag="o0")
                    nc.vector.tensor_copy(out=osb0[:], in_=pj0[:])
                    nc.sync.dma_start(out=out_d[:, c0o : c0o + 512], in_=osb0[:])
                    pj1 = miscp.tile([C, 512], F32, tag="m")
                    nc.tensor.matmul(
                        pj1[:],
                        lhsT=pwaug2[64:97, :],
                        rhs=U2[64:97, :],
                        tile_position=(64, 0),
                    )
                    osb1 = epi.tile([C, 512], F32, tag="o1")
                    nc.vector.tensor_copy(out=osb1[:], in_=pj1[:])
                    nc.sync.dma_start(out=out_d[:, c1o : c1o + 512], in_=osb1[:])

    nc.compile()
    return nc


_NC_CACHE = None


def _get_program():
    global _NC_CACHE
    if _NC_CACHE is None:
        _NC_CACHE = build_program()
    return _NC_CACHE


def kernel(x, norm_w, norm_b, qkv_w, qkv_b, proj_w, proj_b):
    x = np.asarray(x, np.float32)
    norm_w = np.asarray(norm_w, np.float32)
    norm_b = np.asarray(norm_b, np.float32)
    qkv_w = np.asarray(qkv_w, np.float32)
    qkv_b = np.asarray(qkv_b, np.float32)
    proj_w = np.asarray(proj_w, np.float32)
    proj_b = np.asarray(proj_b, np.float32)

    nc = _get_program()

    gs = np.zeros((C, NG), np.float32)
    gs[np.arange(C), np.arange(C) // GS] = 1.0 / GS
    g2 = np.zeros((NG, C), np.float32)
    g2[np.arange(C) // GS, np.arange(C)] = 1.0

    in_maps = []
    for ci in range(N_CORES):
        b, h = ci // NH, ci % NH
        sl = slice(DH * h, DH * (h + 1))
        wqT = qkv_w[sl, :].T
        wkT = qkv_w[C:][sl, :].T
        cpk = np.concatenate(
            [
                np.tile(wqT, (1, 4)),
                np.tile(wkT, (1, 4)),
                qkv_w[2 * C :][sl, :].T,
                np.tile(qkv_b[sl].reshape(DH, 1), (4, 1)),
                gs,
                norm_w.reshape(C, 1),
                norm_b.reshape(C, 1),
            ],
            axis=1,
        )
        in_maps.append(
            {
                "xs": np.ascontiguousarray(x[b].reshape(C, HW)),
                "cpk": np.ascontiguousarray(cpk),
                "pwTa": np.ascontiguousarray(proj_w[:, sl].T),
                "g2": g2,
            }
        )

    res = run_bass_kernel_spmd(nc, in_maps, core_ids=list(range(N_CORES)))

    # unshard: sum per-head partials, add residual + proj bias + v-bias term
    base = proj_b + proj_w @ qkv_b[2 * C :]
    out = np.empty((B, C, HW), np.float32)
    for b in range(B):
        acc = np.zeros((C, HW), np.float32)
        for h in range(NH):
            r = res.results[b * NH + h]
            acc += r["out_p"] / r["den_p"]
        out[b] = acc + x[b].reshape(C, HW) + base[:, None]
    return out.reshape(B, C, 64, 64)


# revision 29
# speedup vs baseline: 1.2248x; 1.2248x over previous
"""AttentionBlock (GroupNorm + single-head-per-core spatial attention + proj)
for Trainium2, 8 NeuronCores.

Sharding: core i handles (batch b = i//4, head h = i%4).  Each core computes
its head's attention output projected through its slice of proj_w's input
channels; the host sums the 4 per-head partials per batch (tensor-parallel
unshard) and adds residual + biases.

Reference semantics (B=2, C=128, H=W=64, heads=4, groups=32, eps=1e-5):
  h   = groupnorm(x) * nw + nb
  qkv = qkv_w @ h + qkv_b          (1x1 conv == channel matmul)
  S   = (q^T k) / sqrt(32); A = softmax(S, axis=t); out = v A^T
  y   = proj_w @ out + proj_b + x

v2 structure (vs. the earlier single-engine-exp version):
  - exp split across ScalarE (exact table exp) and VectorE (Schraudolph
    bit-trick exp: int32(A*S + B) bitcast to float ~= e^S, ~3% per-weight
    err that washes out in the softmax-weighted sums; validated 1.6e-3
    scale-rel worst case with ALL quads approximated).
  - s-chunks processed in PAIRS: each [128,1024] S^T tile = one t-block x
    two s-chunks, halving per-element weight loads; the two AV accumulator
    slabs live at PSUM partitions 0:33 / 64:97 of one bank and run as
    column-tiled concurrent matmuls (tile_position cols 0 / 64).
  - k bias dropped entirely (constant-in-t terms cancel in softmax over t);
    groupnorm folded into q/k/v weights; f32r matmuls via bitcast (no
    separate f32r copy of X).
  - PE warm-up: dummy matmuls during the DMA/stats preamble trip the HAM
    activity monitor so the attention stream runs at 2.4 GHz from the start.
  - softmax denominators ride row 0 of each AV slab (ones column in v^T);
    per-pair both reciprocals computed in one strided 2-partition DVE op.
  - residual, proj_b, and the v-bias term (proj_w @ bv) are added on host.
"""

import sys

sys.path.insert(0, "/opt/trn_rl_repo")

import numpy as np

import concourse.bass as bass
import concourse.tile as tile
from concourse import bacc, mybir
from concourse.bass_utils import run_bass_kernel_spmd

F32 = mybir.dt.float32
F32R = mybir.dt.float32r
I16 = mybir.dt.int16
BF16 = mybir.dt.bfloat16

B, C, HW = 2, 128, 4096
NH, DH = 4, 32
NG, GS = 32, 4  # groups, channels per group
EPS = 1e-5
SCALE = 1.0 / np.sqrt(DH)

# Schraudolph exp in bf16: int16((A*x + B) / 2^16) bit-pattern == bf16(e^x)
# (top 16 bits of the classic int32 f32-bit-pattern trick; ~3% sawtooth err)
A_EXP = float((1 << 23) / np.log(2.0) * SCALE / 65536.0)  # scale folded in
B_EXP = float((127.0 * (1 << 23) - 0.0437 * (1 << 23)) / 65536.0)

N_CORES = 8
N_WARM = 36  # HAM warm-up matmuls during preamble
LAG = 4  # units between exp(u) and AV(u)
# 14 of 32 units per pair go to the DVE Schraudolph path
DVE_UNITS = frozenset(u for u in range(32) if (u % 16) in (1, 3, 5, 7, 9, 11, 13))

MUL = mybir.AluOpType.mult
ADD = mybir.AluOpType.add
SUB = mybir.AluOpType.subtract


def build_program():
    nc = bacc.Bacc("TRN2", target_bir_lowering=False, debug=False)

    def din(name, shape):
        return nc.dram_tensor(name, shape, F32, kind="ExternalInput").ap()

    xs = din("xs", [C, HW])
    cpk = din("cpk", [C, 323])  # packed: wqT4|wkT4|wvT|bq4|gs|nw|nb
    pwTa = din("pwTa", [DH, C])
    g2 = din("g2", [NG, C])
    out_d = nc.dram_tensor("out_p", [C, HW], F32, kind="ExternalOutput").ap()
    den_d = nc.dram_tensor("den_p", [1, HW], F32, kind="ExternalOutput").ap()

    ident = mybir.ActivationFunctionType.Identity
    fcopy = mybir.ActivationFunctionType.Copy
    fexp = mybir.ActivationFunctionType.Exp
    fsqrt = mybir.ActivationFunctionType.Sqrt
    fln = mybir.ActivationFunctionType.Ln

    with tile.TileContext(nc) as tc:
        with (
            tc.tile_pool(name="consts", bufs=1) as consts,
            tc.tile_pool(name="xpool", bufs=1) as xpool,
            tc.tile_pool(name="qk", bufs=1) as qkpool,
            tc.tile_pool(name="vt", bufs=1) as vtpool,
            tc.tile_pool(name="small", bufs=8) as small,
            tc.tile_pool(name="epool", bufs=5) as epool,
            tc.tile_pool(name="epi", bufs=2) as epi,
        ):
            # ---------------- input DMAs ----------------
            cbuf = consts.tile([C, 323], F32)
            nc.scalar.dma_start(cbuf[:], cpk[:])
            c_wqT4 = cbuf[:, 0:128]
            c_wkT4 = cbuf[:, 128:256]
            c_wvT = cbuf[:, 256:288]
            c_bq4 = cbuf[:, 288:289]
            c_gs = cbuf[:, 289:321]
            c_nw = cbuf[:, 321:322]
            c_nb = cbuf[:, 322:323]
            # proj lhsT, twice (partitions 0:33 and 64:97): row 0/64 = runtime
            # addvec, rows 1:33 / 65:97 = pwTa
            pwaug2 = consts.tile([97, C], F32R)
            c_pwTa = consts.tile([DH, C], F32)
            nc.scalar.dma_start(c_pwTa[:], pwTa[:])
            c_g2 = consts.tile([NG, C], F32)
            nc.scalar.dma_start(c_g2[:], g2[:])
            X = xpool.tile([C, HW], F32)
            dma_engs = [nc.sync, nc.scalar, nc.gpsimd, nc.sync]
            for j in range(4):
                dma_engs[j].dma_start(
                    X[:, 1024 * j : 1024 * (j + 1)], xs[:, 1024 * j : 1024 * (j + 1)]
                )
            # f32r copy of X (verifier requires a rounding producer for f32r
            # matmul inputs); split ACT/DVE, pipelined behind the chunk DMAs
            Xr_t = xpool.tile([C, HW], F32R, tag="Xr")
            for j in range(4):
                if j % 2 == 0:
                    nc.scalar.copy(
                        out=Xr_t[:, 1024 * j : 1024 * (j + 1)],
                        in_=X[:, 1024 * j : 1024 * (j + 1)],
                    )
                else:
                    nc.vector.tensor_copy(
                        out=Xr_t[:, 1024 * j : 1024 * (j + 1)],
                        in_=X[:, 1024 * j : 1024 * (j + 1)],
                    )
            Xr = Xr_t[:]

            # small consts
            eps_t = consts.tile([NG, 1], F32)
            nc.vector.memset(eps_t[:], EPS)
            ones_f = consts.tile([128, 1], F32)
            nc.vector.memset(ones_f[:], 1.0)
            warm_0 = consts.tile([128, 512], F32)
            nc.gpsimd.memset(warm_0[:], 0.0)
            warm_w = consts.tile([128, 128], F32R)
            nc.vector.tensor_copy(out=warm_w[:], in_=warm_0[:, 0:128])
            warm_r = consts.tile([128, 512], F32R)
            nc.vector.tensor_copy(out=warm_r[:], in_=warm_0[:])
            tblw = small.tile([NG, 1], F32)

            # per-t-block AV lhsT: cols 0:32 = v, col 32 = ones (denom row)
            v_t = vtpool.tile([128, 32, DH + 1], BF16)
            nc.vector.tensor_copy(
                out=v_t[:, :, DH], in_=ones_f[:, 0:1].to_broadcast([128, 32])
            )
            q_sb = qkpool.tile([128, HW], F32R, tag="q")
            k_sb = qkpool.tile([128, HW], F32R, tag="k")

            with (
                tc.tile_pool(name="warmp", bufs=1, space="PSUM") as warmp,
                tc.tile_pool(name="pp", bufs=1, space="PSUM") as pp,
                tc.tile_pool(name="buildp", bufs=2, space="PSUM") as buildp,
            ):
                # ACT sqrt-table load early (hides under DMA); the Exp set is
                # loaded right after the real sqrt below.
                nc.scalar.activation(out=tblw[:], in_=eps_t[:], func=fsqrt, scale=1.0)

                # HAM warm-up: keep the PE busy through the preamble so the
                # attention stream starts at 2.4 GHz.
                wps = warmp.tile([128, 512], F32)
                for i in range(N_WARM):
                    nc.tensor.matmul(
                        wps[:],
                        lhsT=warm_w[:],
                        rhs=warm_r[:],
                        start=(i == 0),
                        stop=(i == N_WARM - 1),
                    )

                # ---------------- groupnorm stats ----------------
                Xg = X[:].rearrange("c (n f) -> c n f", f=512)
                stats = small.tile([C, 8, 6], F32)
                for i in range(8):
                    nc.vector.bn_stats(out=stats[:, i, :], in_=Xg[:, i, :])
                mv = small.tile([C, 2], F32)
                nc.vector.bn_aggr(out=mv[:], in_=stats[:])
                # mv2 = [mean_c, E[x^2]_c]
                mv2 = small.tile([C, 2], F32)
                nc.vector.tensor_copy(out=mv2[:, 0:1], in_=mv[:, 0:1])
                nc.vector.tensor_tensor(
                    out=mv2[:, 1:2], in0=mv[:, 0:1], in1=mv[:, 0:1], op=MUL
                )
                nc.vector.tensor_tensor(
                    out=mv2[:, 1:2], in0=mv2[:, 1:2], in1=mv[:, 1:2], op=ADD
                )
                gstat_ps = pp.tile([NG, 2], F32, tag="pp")
                nc.tensor.matmul(gstat_ps[:], lhsT=c_gs[:], rhs=mv2[:])
                gstat = small.tile([NG, 2], F32)
                nc.vector.tensor_copy(out=gstat[:], in_=gstat_ps[:])
                varg = small.tile([NG, 1], F32)
                nc.vector.tensor_tensor(
                    out=varg[:], in0=gstat[:, 0:1], in1=gstat[:, 0:1], op=MUL
                )
                nc.vector.tensor_tensor(
                    out=varg[:], in0=gstat[:, 1:2], in1=varg[:], op=SUB
                )
                stdg = small.tile([NG, 1], F32)
                nc.scalar.activation(
                    out=stdg[:], in_=varg[:], func=fsqrt, bias=eps_t[:], scale=1.0
                )
                # switch ACT to the exp table set now (load hides under builds)
                nc.scalar.activation(out=tblw[:], in_=eps_t[:], func=fexp, scale=1.0)
                rstdg = small.tile([NG, 1], F32)
                nc.vector.reciprocal(out=rstdg[:], in_=stdg[:])
                gexp = small.tile([NG, 2], F32)
                nc.vector.tensor_copy(out=gexp[:, 0:1], in_=gstat[:, 0:1])
                nc.vector.tensor_copy(out=gexp[:, 1:2], in_=rstdg[:])
                mrc_ps = pp.tile([C, 2], F32, tag="pp")
                nc.tensor.matmul(mrc_ps[:], lhsT=c_g2[:], rhs=gexp[:])
                mrc = small.tile([C, 2], F32)
                nc.vector.tensor_copy(out=mrc[:], in_=mrc_ps[:])
                # scale_c = rstd_c * nw ; shift_c = nb - mean_c*scale_c
                scale_c = small.tile([C, 1], F32)
                nc.vector.tensor_tensor(
                    out=scale_c[:], in0=mrc[:, 1:2], in1=c_nw[:], op=MUL
                )
                shift_c = small.tile([C, 1], F32)
                nc.vector.tensor_tensor(
                    out=shift_c[:], in0=mrc[:, 0:1], in1=scale_c[:], op=MUL
                )
                nc.vector.tensor_tensor(
                    out=shift_c[:], in0=c_nb[:], in1=shift_c[:], op=SUB
                )
                # folded weights
                wq_f = consts.tile([C, 128], F32R)
                nc.vector.tensor_scalar_mul(out=wq_f[:], in0=c_wqT4[:], scalar1=scale_c[:])
                wk_f = consts.tile([C, 128], F32R)
                nc.vector.tensor_scalar_mul(out=wk_f[:], in0=c_wkT4[:], scalar1=scale_c[:])
                wv_f = consts.tile([C, DH], F32R)
                nc.vector.tensor_scalar_mul(out=wv_f[:], in0=c_wvT[:], scalar1=scale_c[:])
                # adjusted q bias (4x-replicated); k needs no bias at all
                bq_ps = pp.tile([128, 1], F32, tag="pp")
                nc.tensor.matmul(bq_ps[:], lhsT=c_wqT4[:], rhs=shift_c[:])
                bq_f = small.tile([128, 1], F32)
                nc.vector.tensor_tensor(
                    out=bq_f[:], in0=bq_ps[:], in1=c_bq4[:], op=ADD
                )
                # v shift term -> proj addvec rows (partitions 0 and 64)
                vs_ps = pp.tile([DH, 1], F32, tag="pp")
                nc.tensor.matmul(vs_ps[:], lhsT=c_wvT[:], rhs=shift_c[:])
                vsum = small.tile([DH, 1], F32)
                nc.vector.tensor_copy(out=vsum[:], in_=vs_ps[:])
                av_ps = pp.tile([97, C], F32, tag="av")
                nc.tensor.matmul(av_ps[32:33, :], lhsT=vsum[:], rhs=c_pwTa[:])
                nc.tensor.matmul(
                    av_ps[96:97, :], lhsT=vsum[:], rhs=c_pwTa[:],
                    tile_position=(0, 96),
                )
                nc.vector.tensor_copy(out=pwaug2[32:33, :], in_=av_ps[32:33, :])
                nc.vector.tensor_copy(out=pwaug2[96:97, :], in_=av_ps[96:97, :])
                # pwTa rows into both slabs (f32 -> f32r rounding producers)
                nc.vector.tensor_copy(out=pwaug2[0:32, :], in_=c_pwTa[:])
                nc.vector.tensor_copy(out=pwaug2[64:96, :], in_=c_pwTa[:])

                # ---------------- q/k builds ----------------
                for j in range(4):
                    o = 1024 * j
                    qp = buildp.tile([128, 1024], F32, tag="b")
                    nc.tensor.matmul(qp[:, 0:512], lhsT=wq_f[:], rhs=Xr[:, o : o + 512])
                    nc.tensor.matmul(
                        qp[:, 512:1024], lhsT=wq_f[:], rhs=Xr[:, o + 512 : o + 1024]
                    )
                    nc.scalar.activation(
                        out=q_sb[:, o : o + 1024], in_=qp[:], func=ident,
                        bias=bq_f[:], scale=1.0,
                    )
                    kp = buildp.tile([128, 1024], F32, tag="b")
                    nc.tensor.matmul(kp[:, 0:512], lhsT=wk_f[:], rhs=Xr[:, o : o + 512])
                    nc.tensor.matmul(
                        kp[:, 512:1024], lhsT=wk_f[:], rhs=Xr[:, o + 512 : o + 1024]
                    )
                    nc.vector.tensor_copy(out=k_sb[:, o : o + 1024], in_=kp[:])

            # ---------------- attention: 4 pairs of s-chunks ----------------
            with (
                tc.tile_pool(name="sqp", bufs=2, space="PSUM") as sqp,
                tc.tile_pool(name="accp", bufs=1, space="PSUM") as accp,
                tc.tile_pool(name="miscp", bufs=2, space="PSUM") as miscp,
            ):
                for p in range(4):
                    c0o = 1024 * p
                    c1o = 1024 * p + 512
                    acc = accp.tile([128, 512], F32, tag="acc")
                    acc1 = accp.tile([128, 512], F32, tag="acc1")
                    E_tiles = [None] * 32
                    vp = None
                    for it in range(32 + LAG):
                        u = it
                        if u < 32:
                            # q/k are 4x-replicated across partition bands; the
                            # two chunk-matmuls of a unit use DIFFERENT bands
                            # and units alternate band pairs, so any 4
                            # consecutive matmuls cover all 4 row bands ->
                            # 4-up row-tiled concurrency AND the HAM activity
                            # monitor sees a fully-busy array (2.4 GHz).
                            b0 = 2 * (u % 2)
                            b1 = b0 + 1
                            SQ = sqp.tile([128, 1024], F32, tag="sq")
                            nc.tensor.matmul(
                                SQ[:, 0:512],
                                lhsT=k_sb[32 * b0 : 32 * (b0 + 1), 128 * u : 128 * (u + 1)],
                                rhs=q_sb[32 * b0 : 32 * (b0 + 1), c0o : c0o + 512],
                                tile_position=(32 * b0, 0),
                            )
                            nc.tensor.matmul(
                                SQ[:, 512:1024],
                                lhsT=k_sb[32 * b1 : 32 * (b1 + 1), 128 * u : 128 * (u + 1)],
                                rhs=q_sb[32 * b1 : 32 * (b1 + 1), c1o : c1o + 512],
                                tile_position=(32 * b1, 0),
                            )
                            # v^T build rides inside pair 0 (groups of 8 blocks)
                            if p == 0 and u % 8 < 4:
                                g = u // 8
                                if u % 8 == 0:
                                    vp = miscp.tile([128, 256], F32, tag="m")
                                for jj in (2 * (u % 8), 2 * (u % 8) + 1):
                                    tb = 8 * g + jj
                                    nc.tensor.matmul(
                                        vp[:, 32 * jj : 32 * (jj + 1)],
                                        lhsT=Xr[:, 128 * tb : 128 * (tb + 1)],
                                        rhs=wv_f[:],
                                    )
                                if u % 8 == 3:
                                    nc.vector.tensor_copy(
                                        out=v_t[:, 8 * g : 8 * (g + 1), 0:DH],
                                        in_=vp[:].rearrange("p (g d) -> p g d", d=32),
                                    )
                            if u in DVE_UNITS:
                                Ei = epool.tile([128, 1024], I16, tag="ed", bufs=14)
                                nc.vector.tensor_scalar(
                                    Ei[:], SQ[:], A_EXP, B_EXP, MUL, ADD
                                )
                                E_tiles[u] = Ei[:].bitcast(BF16)
                            else:
                                Ea = epool.tile([128, 1024], BF16, tag="ea", bufs=18)
                                nc.scalar.activation(
                                    out=Ea[:], in_=SQ[:], func=fexp, scale=float(SCALE)
                                )
                                E_tiles[u] = Ea[:]
                        ua = it - LAG
                        if 0 <= ua < 32:
                            E = E_tiles[ua]
                            E_tiles[ua] = None
                            # two accumulator slabs in SEPARATE banks (the
                            # start flag's has_written clear is bank-wide);
                            # col positions 0/64 let the two streams overlap
                            nc.tensor.matmul(
                                acc[0:33, :],
                                lhsT=v_t[:, ua, :],
                                rhs=E[:, 0:512],
                                tile_position=(0, 0),
                                start=(ua == 0),
                                stop=(ua == 31),
                            )
                            nc.tensor.matmul(
                                acc1[64:97, :],
                                lhsT=v_t[:, ua, :],
                                rhs=E[:, 512:1024],
                                tile_position=(0, 64),
                                start=(ua == 0),
                                stop=(ua == 31),
                            )

                    # ---------------- pair epilogue ----------------
                    U2 = epi.tile([97, 512], F32R, tag="u2")
                    nc.scalar.activation(
                        out=U2[0:33, :], in_=acc[0:33, :], func=fcopy, scale=1.0
                    )
                    nc.scalar.activation(
                        out=U2[64:97, :], in_=acc1[64:97, :], func=fcopy, scale=1.0
                    )
                    # denominators go to the host (proj is linear, so the
                    # host divides pj by denom during unsharding)
                    nc.sync.dma_start(
                        out=den_d[:, c0o : c0o + 512], in_=U2[32:33, :].bitcast(F32)
                    )
                    nc.sync.dma_start(
                        out=den_d[:, c1o : c1o + 512], in_=U2[96:97, :].bitcast(F32)
                    )
                    pj0 = miscp.tile([C, 512], F32, tag="m")
                    nc.tensor.matmul(
                        pj0[:], lhsT=pwaug2[0:33, :], rhs=U2[0:33, :]
                    )
                    osb0 = epi.tile([C, 512], F32, tag="o0")
                    nc.vector.tensor_copy(out=osb0[:], in_=pj0[:])
                    nc.sync.dma_start(out=out_d[:, c0o : c0o + 512], in_=osb0[:])
                    pj1 = miscp.tile([C, 512], F32, tag="m")
                    nc.tensor.matmul(
                        pj1[:],
                        lhsT=pwaug2[64:97, :],
                        rhs=U2[64:97, :],
                        tile_position=(64, 0),
                    )
                    osb1 = epi.tile([C, 512], F32, tag="o1")
                    nc.vector.tensor_copy(out=osb1[:], in_=pj1[:])
                    nc.sync.dma_start(out=out_d[:, c1o : c1o + 512], in_=osb1[:])

    nc.compile()
    return nc


_NC_CACHE = None


def _get_program():
    global _NC_CACHE
    if _NC_CACHE is None:
        _NC_CACHE = build_program()
    return _NC_CACHE


def kernel(x, norm_w, norm_b, qkv_w, qkv_b, proj_w, proj_b):
    x = np.asarray(x, np.float32)
    norm_w = np.asarray(norm_w, np.float32)
    norm_b = np.asarray(norm_b, np.float32)
    qkv_w = np.asarray(qkv_w, np.float32)
    qkv_b = np.asarray(qkv_b, np.float32)
    proj_w = np.asarray(proj_w, np.float32)
    proj_b = np.asarray(proj_b, np.float32)

    nc = _get_program()

    gs = np.zeros((C, NG), np.float32)
    gs[np.arange(C), np.arange(C) // GS] = 1.0 / GS
    g2 = np.zeros((NG, C), np.float32)
    g2[np.arange(C) // GS, np.arange(C)] = 1.0

    in_maps = []
    for ci in range(N_CORES):
        b, h = ci // NH, ci % NH
        sl = slice(DH * h, DH * (h + 1))
        wqT = qkv_w[sl, :].T
        wkT = qkv_w[C:][sl, :].T
        cpk = np.concatenate(
            [
                np.tile(wqT, (1, 4)),
                np.tile(wkT, (1, 4)),
                qkv_w[2 * C :][sl, :].T,
                np.tile(qkv_b[sl].reshape(DH, 1), (4, 1)),
                gs,
                norm_w.reshape(C, 1),
                norm_b.reshape(C, 1),
            ],
            axis=1,
        )
        in_maps.append(
            {
                "xs": np.ascontiguousarray(x[b].reshape(C, HW)),
                "cpk": np.ascontiguousarray(cpk),
                "pwTa": np.ascontiguousarray(proj_w[:, sl].T),
                "g2": g2,
            }
        )

    res = run_bass_kernel_spmd(nc, in_maps, core_ids=list(range(N_CORES)))

    # unshard: sum per-head partials, add residual + proj bias + v-bias term
    base = proj_b + proj_w @ qkv_b[2 * C :]
    out = np.empty((B, C, HW), np.float32)
    for b in range(B):
        acc = np.zeros((C, HW), np.float32)
        for h in range(NH):
            r = res.results[b * NH + h]
            acc += r["out_p"] / r["den_p"]
        out[b] = acc + x[b].reshape(C, HW) + base[:, None]
    return out.reshape(B, C, 64, 64)


# revision 30
# speedup vs baseline: 1.5451x; 1.2615x over previous
"""AttentionBlock (GroupNorm + single-head-per-core spatial attention + proj)
for Trainium2, 8 NeuronCores.

Sharding: core i handles (batch b = i//4, head h = i%4).  Each core computes
its head's attention output projected through its slice of proj_w's input
channels; the host sums the 4 per-head partials per batch (tensor-parallel
unshard) and adds residual + biases.

Reference semantics (B=2, C=128, H=W=64, heads=4, groups=32, eps=1e-5):
  h   = groupnorm(x) * nw + nb
  qkv = qkv_w @ h + qkv_b          (1x1 conv == channel matmul)
  S   = (q^T k) / sqrt(32); A = softmax(S, axis=t); out = v A^T
  y   = proj_w @ out + proj_b + x

v2 structure (vs. the earlier single-engine-exp version):
  - exp split across ScalarE (exact table exp) and VectorE (Schraudolph
    bit-trick exp: int32(A*S + B) bitcast to float ~= e^S, ~3% per-weight
    err that washes out in the softmax-weighted sums; validated 1.6e-3
    scale-rel worst case with ALL quads approximated).
  - s-chunks processed in PAIRS: each [128,1024] S^T tile = one t-block x
    two s-chunks, halving per-element weight loads; the two AV accumulator
    slabs live at PSUM partitions 0:33 / 64:97 of one bank and run as
    column-tiled concurrent matmuls (tile_position cols 0 / 64).
  - k bias dropped entirely (constant-in-t terms cancel in softmax over t);
    groupnorm folded into q/k/v weights; f32r matmuls via bitcast (no
    separate f32r copy of X).
  - PE warm-up: dummy matmuls during the DMA/stats preamble trip the HAM
    activity monitor so the attention stream runs at 2.4 GHz from the start.
  - softmax denominators ride row 0 of each AV slab (ones column in v^T);
    per-pair both reciprocals computed in one strided 2-partition DVE op.
  - residual, proj_b, and the v-bias term (proj_w @ bv) are added on host.
"""

import sys

sys.path.insert(0, "/opt/trn_rl_repo")

import numpy as np

import concourse.bass as bass
import concourse.tile as tile
from concourse import bacc, mybir
from concourse.bass_utils import run_bass_kernel_spmd

F32 = mybir.dt.float32
F32R = mybir.dt.float32r
I16 = mybir.dt.int16
BF16 = mybir.dt.bfloat16

B, C, HW = 2, 128, 4096
NH, DH = 4, 32
NG, GS = 32, 4  # groups, channels per group
EPS = 1e-5
SCALE = 1.0 / np.sqrt(DH)

# Schraudolph exp in bf16: int16((A*x + B) / 2^16) bit-pattern == bf16(e^x)
# (top 16 bits of the classic int32 f32-bit-pattern trick; ~3% sawtooth err)
A_EXP = float((1 << 23) / np.log(2.0) * SCALE / 65536.0)  # scale folded in
B_EXP = float((127.0 * (1 << 23) - 0.0437 * (1 << 23)) / 65536.0)

N_CORES = 8
N_WARM = 36  # HAM warm-up matmuls during preamble
GLAG = 2  # 2-unit groups between exp and AV consumption
# 14 of 32 units per pair go to the DVE Schraudolph path
DVE_UNITS = frozenset(u for u in range(32) if (u % 16) in (1, 3, 5, 7, 9, 11, 13))

MUL = mybir.AluOpType.mult
ADD = mybir.AluOpType.add
SUB = mybir.AluOpType.subtract


def build_program():
    nc = bacc.Bacc("TRN2", target_bir_lowering=False, debug=False)

    def din(name, shape):
        return nc.dram_tensor(name, shape, F32, kind="ExternalInput").ap()

    xs = din("xs", [C, HW])
    cpk = din("cpk", [C, 323])  # packed: wqT4|wkT4|wvT|bq4|gs|nw|nb
    pwTa = din("pwTa", [DH, C])
    g2 = din("g2", [NG, C])
    out_d = nc.dram_tensor("out_p", [C, HW], F32, kind="ExternalOutput").ap()
    den_d = nc.dram_tensor("den_p", [1, HW], F32, kind="ExternalOutput").ap()

    ident = mybir.ActivationFunctionType.Identity
    fcopy = mybir.ActivationFunctionType.Copy
    fexp = mybir.ActivationFunctionType.Exp
    fsqrt = mybir.ActivationFunctionType.Sqrt
    fln = mybir.ActivationFunctionType.Ln

    with tile.TileContext(nc) as tc:
        with (
            tc.tile_pool(name="consts", bufs=1) as consts,
            tc.tile_pool(name="xpool", bufs=1) as xpool,
            tc.tile_pool(name="qk", bufs=1) as qkpool,
            tc.tile_pool(name="vt", bufs=1) as vtpool,
            tc.tile_pool(name="small", bufs=8) as small,
            tc.tile_pool(name="epool", bufs=5) as epool,
            tc.tile_pool(name="epi", bufs=2) as epi,
        ):
            # ---------------- input DMAs ----------------
            cbuf = consts.tile([C, 323], F32)
            nc.scalar.dma_start(cbuf[:], cpk[:])
            c_wqT4 = cbuf[:, 0:128]
            c_wkT4 = cbuf[:, 128:256]
            c_wvT = cbuf[:, 256:288]
            c_bq4 = cbuf[:, 288:289]
            c_gs = cbuf[:, 289:321]
            c_nw = cbuf[:, 321:322]
            c_nb = cbuf[:, 322:323]
            # proj lhsT, twice (partitions 0:33 and 64:97): row 0/64 = runtime
            # addvec, rows 1:33 / 65:97 = pwTa
            pwaug2 = consts.tile([97, C], F32R)
            c_pwTa = consts.tile([DH, C], F32)
            nc.scalar.dma_start(c_pwTa[:], pwTa[:])
            c_g2 = consts.tile([NG, C], F32)
            nc.scalar.dma_start(c_g2[:], g2[:])
            X = xpool.tile([C, HW], F32)
            dma_engs = [nc.sync, nc.scalar, nc.gpsimd, nc.sync]
            for j in range(4):
                dma_engs[j].dma_start(
                    X[:, 1024 * j : 1024 * (j + 1)], xs[:, 1024 * j : 1024 * (j + 1)]
                )
            # f32r copy of X (verifier requires a rounding producer for f32r
            # matmul inputs); split ACT/DVE, pipelined behind the chunk DMAs
            Xr_t = xpool.tile([C, HW], F32R, tag="Xr")
            for j in range(4):
                if j % 2 == 0:
                    nc.scalar.copy(
                        out=Xr_t[:, 1024 * j : 1024 * (j + 1)],
                        in_=X[:, 1024 * j : 1024 * (j + 1)],
                    )
                else:
                    nc.vector.tensor_copy(
                        out=Xr_t[:, 1024 * j : 1024 * (j + 1)],
                        in_=X[:, 1024 * j : 1024 * (j + 1)],
                    )
            Xr = Xr_t[:]

            # small consts
            eps_t = consts.tile([NG, 1], F32)
            nc.vector.memset(eps_t[:], EPS)
            ones_f = consts.tile([128, 1], F32)
            nc.vector.memset(ones_f[:], 1.0)
            warm_0 = consts.tile([128, 512], F32)
            nc.gpsimd.memset(warm_0[:], 0.0)
            warm_w = consts.tile([128, 128], F32R)
            nc.vector.tensor_copy(out=warm_w[:], in_=warm_0[:, 0:128])
            warm_r = consts.tile([128, 512], F32R)
            nc.vector.tensor_copy(out=warm_r[:], in_=warm_0[:])
            tblw = small.tile([NG, 1], F32)

            # per-t-block AV lhsT padded to M=64: cols 0:32 = v, col 32 =
            # ones (denom row), cols 33:64 zero (full-array HAM activity)
            v_t = vtpool.tile([128, 32, 64], BF16)
            nc.vector.memset(v_t[:], 0.0)
            nc.vector.tensor_copy(
                out=v_t[:, :, DH], in_=ones_f[:, 0:1].to_broadcast([128, 32])
            )
            q_sb = qkpool.tile([128, HW], F32R, tag="q")
            k_sb = qkpool.tile([128, HW], F32R, tag="k")

            with (
                tc.tile_pool(name="warmp", bufs=1, space="PSUM") as warmp,
                tc.tile_pool(name="pp", bufs=1, space="PSUM") as pp,
                tc.tile_pool(name="buildp", bufs=2, space="PSUM") as buildp,
            ):
                # ACT sqrt-table load early (hides under DMA); the Exp set is
                # loaded right after the real sqrt below.
                nc.scalar.activation(out=tblw[:], in_=eps_t[:], func=fsqrt, scale=1.0)

                # HAM warm-up: keep the PE busy through the preamble so the
                # attention stream starts at 2.4 GHz.
                wps = warmp.tile([128, 512], F32)
                for i in range(N_WARM):
                    nc.tensor.matmul(
                        wps[:],
                        lhsT=warm_w[:],
                        rhs=warm_r[:],
                        start=(i == 0),
                        stop=(i == N_WARM - 1),
                    )

                # ---------------- groupnorm stats ----------------
                Xg = X[:].rearrange("c (n f) -> c n f", f=512)
                stats = small.tile([C, 8, 6], F32)
                for i in range(8):
                    nc.vector.bn_stats(out=stats[:, i, :], in_=Xg[:, i, :])
                mv = small.tile([C, 2], F32)
                nc.vector.bn_aggr(out=mv[:], in_=stats[:])
                # mv2 = [mean_c, E[x^2]_c]
                mv2 = small.tile([C, 2], F32)
                nc.vector.tensor_copy(out=mv2[:, 0:1], in_=mv[:, 0:1])
                nc.vector.tensor_tensor(
                    out=mv2[:, 1:2], in0=mv[:, 0:1], in1=mv[:, 0:1], op=MUL
                )
                nc.vector.tensor_tensor(
                    out=mv2[:, 1:2], in0=mv2[:, 1:2], in1=mv[:, 1:2], op=ADD
                )
                gstat_ps = pp.tile([NG, 2], F32, tag="pp")
                nc.tensor.matmul(gstat_ps[:], lhsT=c_gs[:], rhs=mv2[:])
                gstat = small.tile([NG, 2], F32)
                nc.vector.tensor_copy(out=gstat[:], in_=gstat_ps[:])
                varg = small.tile([NG, 1], F32)
                nc.vector.tensor_tensor(
                    out=varg[:], in0=gstat[:, 0:1], in1=gstat[:, 0:1], op=MUL
                )
                nc.vector.tensor_tensor(
                    out=varg[:], in0=gstat[:, 1:2], in1=varg[:], op=SUB
                )
                stdg = small.tile([NG, 1], F32)
                nc.scalar.activation(
                    out=stdg[:], in_=varg[:], func=fsqrt, bias=eps_t[:], scale=1.0
                )
                # switch ACT to the exp table set now (load hides under builds)
                nc.scalar.activation(out=tblw[:], in_=eps_t[:], func=fexp, scale=1.0)
                rstdg = small.tile([NG, 1], F32)
                nc.vector.reciprocal(out=rstdg[:], in_=stdg[:])
                gexp = small.tile([NG, 2], F32)
                nc.vector.tensor_copy(out=gexp[:, 0:1], in_=gstat[:, 0:1])
                nc.vector.tensor_copy(out=gexp[:, 1:2], in_=rstdg[:])
                mrc_ps = pp.tile([C, 2], F32, tag="pp")
                nc.tensor.matmul(mrc_ps[:], lhsT=c_g2[:], rhs=gexp[:])
                mrc = small.tile([C, 2], F32)
                nc.vector.tensor_copy(out=mrc[:], in_=mrc_ps[:])
                # scale_c = rstd_c * nw ; shift_c = nb - mean_c*scale_c
                scale_c = small.tile([C, 1], F32)
                nc.vector.tensor_tensor(
                    out=scale_c[:], in0=mrc[:, 1:2], in1=c_nw[:], op=MUL
                )
                shift_c = small.tile([C, 1], F32)
                nc.vector.tensor_tensor(
                    out=shift_c[:], in0=mrc[:, 0:1], in1=scale_c[:], op=MUL
                )
                nc.vector.tensor_tensor(
                    out=shift_c[:], in0=c_nb[:], in1=shift_c[:], op=SUB
                )
                # folded weights
                wq_f = consts.tile([C, 128], F32R)
                nc.vector.tensor_scalar_mul(out=wq_f[:], in0=c_wqT4[:], scalar1=scale_c[:])
                wk_f = consts.tile([C, 128], F32R)
                nc.vector.tensor_scalar_mul(out=wk_f[:], in0=c_wkT4[:], scalar1=scale_c[:])
                wv_f = consts.tile([C, DH], F32R)
                nc.vector.tensor_scalar_mul(out=wv_f[:], in0=c_wvT[:], scalar1=scale_c[:])
                # adjusted q bias (4x-replicated); k needs no bias at all
                bq_ps = pp.tile([128, 1], F32, tag="pp")
                nc.tensor.matmul(bq_ps[:], lhsT=c_wqT4[:], rhs=shift_c[:])
                bq_f = small.tile([128, 1], F32)
                nc.vector.tensor_tensor(
                    out=bq_f[:], in0=bq_ps[:], in1=c_bq4[:], op=ADD
                )
                # v shift term -> proj addvec rows (partitions 0 and 64)
                vs_ps = pp.tile([DH, 1], F32, tag="pp")
                nc.tensor.matmul(vs_ps[:], lhsT=c_wvT[:], rhs=shift_c[:])
                vsum = small.tile([DH, 1], F32)
                nc.vector.tensor_copy(out=vsum[:], in_=vs_ps[:])
                av_ps = pp.tile([97, C], F32, tag="av")
                nc.tensor.matmul(av_ps[32:33, :], lhsT=vsum[:], rhs=c_pwTa[:])
                nc.tensor.matmul(
                    av_ps[96:97, :], lhsT=vsum[:], rhs=c_pwTa[:],
                    tile_position=(0, 96),
                )
                nc.vector.tensor_copy(out=pwaug2[32:33, :], in_=av_ps[32:33, :])
                nc.vector.tensor_copy(out=pwaug2[96:97, :], in_=av_ps[96:97, :])
                # pwTa rows into both slabs (f32 -> f32r rounding producers)
                nc.vector.tensor_copy(out=pwaug2[0:32, :], in_=c_pwTa[:])
                nc.vector.tensor_copy(out=pwaug2[64:96, :], in_=c_pwTa[:])

                # ---------------- q/k builds ----------------
                for j in range(4):
                    o = 1024 * j
                    qp = buildp.tile([128, 1024], F32, tag="b")
                    nc.tensor.matmul(qp[:, 0:512], lhsT=wq_f[:], rhs=Xr[:, o : o + 512])
                    nc.tensor.matmul(
                        qp[:, 512:1024], lhsT=wq_f[:], rhs=Xr[:, o + 512 : o + 1024]
                    )
                    nc.scalar.activation(
                        out=q_sb[:, o : o + 1024], in_=qp[:], func=ident,
                        bias=bq_f[:], scale=1.0,
                    )
                    kp = buildp.tile([128, 1024], F32, tag="b")
                    nc.tensor.matmul(kp[:, 0:512], lhsT=wk_f[:], rhs=Xr[:, o : o + 512])
                    nc.tensor.matmul(
                        kp[:, 512:1024], lhsT=wk_f[:], rhs=Xr[:, o + 512 : o + 1024]
                    )
                    nc.vector.tensor_copy(out=k_sb[:, o : o + 1024], in_=kp[:])

            # ---------------- attention: 4 pairs of s-chunks ----------------
            # Groups of 2 units: 4 S-matmuls covering all 4 row bands run
            # concurrently (row tiling), then 2 AV pairs whose M=64 padded
            # lhsT lights the full array - keeps the HAM activity monitor
            # busy so the PE stays at 2.4 GHz.
            with (
                tc.tile_pool(name="sqp", bufs=3, space="PSUM") as sqp,
                tc.tile_pool(name="accp", bufs=1, space="PSUM") as accp,
            ):
                for p in range(4):
                    c0o = 1024 * p
                    c1o = 1024 * p + 512
                    acc = accp.tile([128, 512], F32, tag="acc")
                    acc1 = accp.tile([128, 512], F32, tag="acc1")
                    E_tiles = [None] * 32
                    for it in range(16 + GLAG):
                        g = it
                        if g < 16:
                            vp = None
                            if p == 0 and g < 8:
                                vp = sqp.tile([128, 128], F32, tag="sq")
                            for ui, u in enumerate((2 * g, 2 * g + 1)):
                                b0 = 2 * ui
                                b1 = b0 + 1
                                SQ = sqp.tile([128, 1024], F32, tag="sq")
                                nc.tensor.matmul(
                                    SQ[:, 0:512],
                                    lhsT=k_sb[32 * b0 : 32 * (b0 + 1), 128 * u : 128 * (u + 1)],
                                    rhs=q_sb[32 * b0 : 32 * (b0 + 1), c0o : c0o + 512],
                                    tile_position=(32 * b0, 0),
                                )
                                nc.tensor.matmul(
                                    SQ[:, 512:1024],
                                    lhsT=k_sb[32 * b1 : 32 * (b1 + 1), 128 * u : 128 * (u + 1)],
                                    rhs=q_sb[32 * b1 : 32 * (b1 + 1), c1o : c1o + 512],
                                    tile_position=(32 * b1, 0),
                                )
                                if vp is not None:
                                    # v^T build rides along (blocks 4g..4g+3)
                                    for jj in range(2):
                                        tb = 4 * g + 2 * ui + jj
                                        nc.tensor.matmul(
                                            vp[:, 32 * (2 * ui + jj) : 32 * (2 * ui + jj + 1)],
                                            lhsT=Xr[:, 128 * tb : 128 * (tb + 1)],
                                            rhs=wv_f[:],
                                        )
                                if u in DVE_UNITS:
                                    Ei = epool.tile([128, 1024], I16, tag="ed", bufs=14)
                                    nc.vector.tensor_scalar(
                                        Ei[:], SQ[:], A_EXP, B_EXP, MUL, ADD
                                    )
                                    E_tiles[u] = Ei[:].bitcast(BF16)
                                else:
                                    Ea = epool.tile([128, 1024], BF16, tag="ea", bufs=18)
                                    nc.scalar.activation(
                                        out=Ea[:], in_=SQ[:], func=fexp, scale=float(SCALE)
                                    )
                                    E_tiles[u] = Ea[:]
                            if vp is not None:
                                nc.vector.tensor_copy(
                                    out=v_t[:, 4 * g : 4 * (g + 1), 0:DH],
                                    in_=vp[:].rearrange("p (n d) -> p n d", d=32),
                                )
                        ga = it - GLAG
                        if 0 <= ga < 16:
                            for ua in (2 * ga, 2 * ga + 1):
                                E = E_tiles[ua]
                                E_tiles[ua] = None
                                # two accumulator slabs in SEPARATE banks (the
                                # start flag's has_written clear is bank-wide);
                                # col positions 0/64 + M=64 pad -> full-array
                                # concurrent accumulation streams
                                nc.tensor.matmul(
                                    acc[0:64, :],
                                    lhsT=v_t[:, ua, :],
                                    rhs=E[:, 0:512],
                                    tile_position=(0, 0),
                                    start=(ua == 0),
                                    stop=(ua == 31),
                                )
                                nc.tensor.matmul(
                                    acc1[64:128, :],
                                    lhsT=v_t[:, ua, :],
                                    rhs=E[:, 512:1024],
                                    tile_position=(0, 64),
                                    start=(ua == 0),
                                    stop=(ua == 31),
                                )

                    # ---------------- pair epilogue ----------------
                    U2 = epi.tile([97, 512], F32R, tag="u2")
                    nc.scalar.activation(
                        out=U2[0:33, :], in_=acc[0:33, :], func=fcopy, scale=1.0
                    )
                    nc.scalar.activation(
                        out=U2[64:97, :], in_=acc1[64:97, :], func=fcopy, scale=1.0
                    )
                    # denominators go to the host (proj is linear, so the
                    # host divides pj by denom during unsharding)
                    nc.sync.dma_start(
                        out=den_d[:, c0o : c0o + 512], in_=U2[32:33, :].bitcast(F32)
                    )
                    nc.sync.dma_start(
                        out=den_d[:, c1o : c1o + 512], in_=U2[96:97, :].bitcast(F32)
                    )
                    pj0 = accp.tile([C, 512], F32, tag="acc")
                    nc.tensor.matmul(
                        pj0[:], lhsT=pwaug2[0:33, :], rhs=U2[0:33, :]
                    )
                    osb0 = epi.tile([C, 512], F32, tag="o0")
                    nc.vector.tensor_copy(out=osb0[:], in_=pj0[:])
                    nc.sync.dma_start(out=out_d[:, c0o : c0o + 512], in_=osb0[:])
                    pj1 = accp.tile([C, 512], F32, tag="acc1")
                    nc.tensor.matmul(
                        pj1[:],
                        lhsT=pwaug2[64:97, :],
                        rhs=U2[64:97, :],
                        tile_position=(64, 0),
                    )
                    osb1 = epi.tile([C, 512], F32, tag="o1")
                    nc.vector.tensor_copy(out=osb1[:], in_=pj1[:])
                    nc.sync.dma_start(out=out_d[:, c1o : c1o + 512], in_=osb1[:])

    nc.compile()
    return nc


_NC_CACHE = None


def _get_program():
    global _NC_CACHE
    if _NC_CACHE is None:
        _NC_CACHE = build_program()
    return _NC_CACHE


def kernel(x, norm_w, norm_b, qkv_w, qkv_b, proj_w, proj_b):
    x = np.asarray(x, np.float32)
    norm_w = np.asarray(norm_w, np.float32)
    norm_b = np.asarray(norm_b, np.float32)
    qkv_w = np.asarray(qkv_w, np.float32)
    qkv_b = np.asarray(qkv_b, np.float32)
    proj_w = np.asarray(proj_w, np.float32)
    proj_b = np.asarray(proj_b, np.float32)

    nc = _get_program()

    gs = np.zeros((C, NG), np.float32)
    gs[np.arange(C), np.arange(C) // GS] = 1.0 / GS
    g2 = np.zeros((NG, C), np.float32)
    g2[np.arange(C) // GS, np.arange(C)] = 1.0

    in_maps = []
    for ci in range(N_CORES):
        b, h = ci // NH, ci % NH
        sl = slice(DH * h, DH * (h + 1))
        wqT = qkv_w[sl, :].T
        wkT = qkv_w[C:][sl, :].T
        cpk = np.concatenate(
            [
                np.tile(wqT, (1, 4)),
                np.tile(wkT, (1, 4)),
                qkv_w[2 * C :][sl, :].T,
                np.tile(qkv_b[sl].reshape(DH, 1), (4, 1)),
                gs,
                norm_w.reshape(C, 1),
                norm_b.reshape(C, 1),
            ],
            axis=1,
        )
        in_maps.append(
            {
                "xs": np.ascontiguousarray(x[b].reshape(C, HW)),
                "cpk": np.ascontiguousarray(cpk),
                "pwTa": np.ascontiguousarray(proj_w[:, sl].T),
                "g2": g2,
            }
        )

    res = run_bass_kernel_spmd(nc, in_maps, core_ids=list(range(N_CORES)))

    # unshard: sum per-head partials, add residual + proj bias + v-bias term
    base = proj_b + proj_w @ qkv_b[2 * C :]
    out = np.empty((B, C, HW), np.float32)
    for b in range(B):
        acc = np.zeros((C, HW), np.float32)
        for h in range(NH):
            r = res.results[b * NH + h]
            acc += r["out_p"] / r["den_p"]
        out[b] = acc + x[b].reshape(C, HW) + base[:, None]
    return out.reshape(B, C, 64, 64)


# revision 31
# speedup vs baseline: 1.5749x; 1.0192x over previous
"""AttentionBlock (GroupNorm + single-head-per-core spatial attention + proj)
for Trainium2, 8 NeuronCores.

Sharding: core i handles (batch b = i//4, head h = i%4).  Each core computes
its head's attention output projected through its slice of proj_w's input
channels; the host sums the 4 per-head partials per batch (tensor-parallel
unshard) and adds residual + biases.

Reference semantics (B=2, C=128, H=W=64, heads=4, groups=32, eps=1e-5):
  h   = groupnorm(x) * nw + nb
  qkv = qkv_w @ h + qkv_b          (1x1 conv == channel matmul)
  S   = (q^T k) / sqrt(32); A = softmax(S, axis=t); out = v A^T
  y   = proj_w @ out + proj_b + x

v2 structure (vs. the earlier single-engine-exp version):
  - exp split across ScalarE (exact table exp) and VectorE (Schraudolph
    bit-trick exp: int32(A*S + B) bitcast to float ~= e^S, ~3% per-weight
    err that washes out in the softmax-weighted sums; validated 1.6e-3
    scale-rel worst case with ALL quads approximated).
  - s-chunks processed in PAIRS: each [128,1024] S^T tile = one t-block x
    two s-chunks, halving per-element weight loads; the two AV accumulator
    slabs live at PSUM partitions 0:33 / 64:97 of one bank and run as
    column-tiled concurrent matmuls (tile_position cols 0 / 64).
  - k bias dropped entirely (constant-in-t terms cancel in softmax over t);
    groupnorm folded into q/k/v weights; f32r matmuls via bitcast (no
    separate f32r copy of X).
  - PE warm-up: dummy matmuls during the DMA/stats preamble trip the HAM
    activity monitor so the attention stream runs at 2.4 GHz from the start.
  - softmax denominators ride row 0 of each AV slab (ones column in v^T);
    per-pair both reciprocals computed in one strided 2-partition DVE op.
  - residual, proj_b, and the v-bias term (proj_w @ bv) are added on host.
"""

import sys

sys.path.insert(0, "/opt/trn_rl_repo")

import numpy as np

import concourse.bass as bass
import concourse.tile as tile
from concourse import bacc, mybir
from concourse.bass_utils import run_bass_kernel_spmd

F32 = mybir.dt.float32
F32R = mybir.dt.float32r
I16 = mybir.dt.int16
BF16 = mybir.dt.bfloat16

B, C, HW = 2, 128, 4096
NH, DH = 4, 32
NG, GS = 32, 4  # groups, channels per group
EPS = 1e-5
SCALE = 1.0 / np.sqrt(DH)

# Schraudolph exp in bf16: int16((A*x + B) / 2^16) bit-pattern == bf16(e^x)
# (top 16 bits of the classic int32 f32-bit-pattern trick; ~3% sawtooth err)
A_EXP = float((1 << 23) / np.log(2.0) * (SCALE / 4.0) / 65536.0)  # /4: replica-sum S
B_EXP = float((127.0 * (1 << 23) - 0.0437 * (1 << 23)) / 65536.0)

N_CORES = 8
N_WARM = 36  # HAM warm-up matmuls during preamble
GLAG = 2  # 2-unit groups between exp and AV consumption
# 14 of 32 units per pair go to the DVE Schraudolph path
DVE_UNITS = frozenset(u for u in range(32) if (u % 16) in (1, 3, 5, 7, 9, 11, 13))

MUL = mybir.AluOpType.mult
ADD = mybir.AluOpType.add
SUB = mybir.AluOpType.subtract


def build_program():
    nc = bacc.Bacc("TRN2", target_bir_lowering=False, debug=False)

    def din(name, shape):
        return nc.dram_tensor(name, shape, F32, kind="ExternalInput").ap()

    xs = din("xs", [C, HW])
    cpk = din("cpk", [C, 323])  # packed: wqT4|wkT4|wvT|bq4|gs|nw|nb
    pwTa = din("pwTa", [DH, C])
    g2 = din("g2", [NG, C])
    out_d = nc.dram_tensor("out_p", [C, HW], F32, kind="ExternalOutput").ap()
    den_d = nc.dram_tensor("den_p", [1, HW], F32, kind="ExternalOutput").ap()

    ident = mybir.ActivationFunctionType.Identity
    fcopy = mybir.ActivationFunctionType.Copy
    fexp = mybir.ActivationFunctionType.Exp
    fsqrt = mybir.ActivationFunctionType.Sqrt
    fln = mybir.ActivationFunctionType.Ln

    with tile.TileContext(nc) as tc:
        with (
            tc.tile_pool(name="consts", bufs=1) as consts,
            tc.tile_pool(name="xpool", bufs=1) as xpool,
            tc.tile_pool(name="qk", bufs=1) as qkpool,
            tc.tile_pool(name="vt", bufs=1) as vtpool,
            tc.tile_pool(name="small", bufs=8) as small,
            tc.tile_pool(name="epool", bufs=5) as epool,
            tc.tile_pool(name="epi", bufs=2) as epi,
        ):
            # ---------------- input DMAs ----------------
            cbuf = consts.tile([C, 323], F32)
            nc.scalar.dma_start(cbuf[:], cpk[:])
            c_wqT4 = cbuf[:, 0:128]
            c_wkT4 = cbuf[:, 128:256]
            c_wvT = cbuf[:, 256:288]
            c_bq4 = cbuf[:, 288:289]
            c_gs = cbuf[:, 289:321]
            c_nw = cbuf[:, 321:322]
            c_nb = cbuf[:, 322:323]
            # proj lhsT, twice (partitions 0:33 and 64:97): row 0/64 = runtime
            # addvec, rows 1:33 / 65:97 = pwTa
            pwaug2 = consts.tile([97, C], F32R)
            c_pwTa = consts.tile([DH, C], F32)
            nc.scalar.dma_start(c_pwTa[:], pwTa[:])
            c_g2 = consts.tile([NG, C], F32)
            nc.scalar.dma_start(c_g2[:], g2[:])
            X = xpool.tile([C, HW], F32)
            dma_engs = [nc.sync, nc.scalar, nc.gpsimd, nc.sync]
            for j in range(4):
                dma_engs[j].dma_start(
                    X[:, 1024 * j : 1024 * (j + 1)], xs[:, 1024 * j : 1024 * (j + 1)]
                )
            # f32r copy of X (verifier requires a rounding producer for f32r
            # matmul inputs); split ACT/DVE, pipelined behind the chunk DMAs
            Xr_t = xpool.tile([C, HW], F32R, tag="Xr")
            for j in range(4):
                if j % 2 == 0:
                    nc.scalar.copy(
                        out=Xr_t[:, 1024 * j : 1024 * (j + 1)],
                        in_=X[:, 1024 * j : 1024 * (j + 1)],
                    )
                else:
                    nc.vector.tensor_copy(
                        out=Xr_t[:, 1024 * j : 1024 * (j + 1)],
                        in_=X[:, 1024 * j : 1024 * (j + 1)],
                    )
            Xr = Xr_t[:]

            # small consts
            eps_t = consts.tile([NG, 1], F32)
            nc.vector.memset(eps_t[:], EPS)
            ones_f = consts.tile([128, 1], F32)
            nc.vector.memset(ones_f[:], 1.0)
            warm_0 = consts.tile([128, 512], F32)
            nc.gpsimd.memset(warm_0[:], 0.0)
            warm_w = consts.tile([128, 128], F32R)
            nc.vector.tensor_copy(out=warm_w[:], in_=warm_0[:, 0:128])
            warm_r = consts.tile([128, 512], F32R)
            nc.vector.tensor_copy(out=warm_r[:], in_=warm_0[:])
            tblw = small.tile([NG, 1], F32)

            # per-t-block AV lhsT padded to M=64: cols 0:32 = v, col 32 =
            # ones (denom row), cols 33:64 zero (full-array HAM activity)
            v_t = vtpool.tile([128, 32, 64], BF16)
            nc.vector.memset(v_t[:], 0.0)
            nc.vector.tensor_copy(
                out=v_t[:, :, DH], in_=ones_f[:, 0:1].to_broadcast([128, 32])
            )
            q_sb = qkpool.tile([128, HW], F32R, tag="q")
            k_sb = qkpool.tile([128, HW], F32R, tag="k")

            with (
                tc.tile_pool(name="warmp", bufs=1, space="PSUM") as warmp,
                tc.tile_pool(name="pp", bufs=1, space="PSUM") as pp,
                tc.tile_pool(name="buildp", bufs=2, space="PSUM") as buildp,
            ):
                # ACT sqrt-table load early (hides under DMA); the Exp set is
                # loaded right after the real sqrt below.
                nc.scalar.activation(out=tblw[:], in_=eps_t[:], func=fsqrt, scale=1.0)

                # HAM warm-up: keep the PE busy through the preamble so the
                # attention stream starts at 2.4 GHz.
                wps = warmp.tile([128, 512], F32)
                for i in range(N_WARM):
                    nc.tensor.matmul(
                        wps[:],
                        lhsT=warm_w[:],
                        rhs=warm_r[:],
                        start=(i == 0),
                        stop=(i == N_WARM - 1),
                    )

                # ---------------- groupnorm stats ----------------
                Xg = X[:].rearrange("c (n f) -> c n f", f=512)
                stats = small.tile([C, 8, 6], F32)
                for i in range(8):
                    nc.vector.bn_stats(out=stats[:, i, :], in_=Xg[:, i, :])
                mv = small.tile([C, 2], F32)
                nc.vector.bn_aggr(out=mv[:], in_=stats[:])
                # mv2 = [mean_c, E[x^2]_c]
                mv2 = small.tile([C, 2], F32)
                nc.vector.tensor_copy(out=mv2[:, 0:1], in_=mv[:, 0:1])
                nc.vector.tensor_tensor(
                    out=mv2[:, 1:2], in0=mv[:, 0:1], in1=mv[:, 0:1], op=MUL
                )
                nc.vector.tensor_tensor(
                    out=mv2[:, 1:2], in0=mv2[:, 1:2], in1=mv[:, 1:2], op=ADD
                )
                gstat_ps = pp.tile([NG, 2], F32, tag="pp")
                nc.tensor.matmul(gstat_ps[:], lhsT=c_gs[:], rhs=mv2[:])
                gstat = small.tile([NG, 2], F32)
                nc.vector.tensor_copy(out=gstat[:], in_=gstat_ps[:])
                varg = small.tile([NG, 1], F32)
                nc.vector.tensor_tensor(
                    out=varg[:], in0=gstat[:, 0:1], in1=gstat[:, 0:1], op=MUL
                )
                nc.vector.tensor_tensor(
                    out=varg[:], in0=gstat[:, 1:2], in1=varg[:], op=SUB
                )
                stdg = small.tile([NG, 1], F32)
                nc.scalar.activation(
                    out=stdg[:], in_=varg[:], func=fsqrt, bias=eps_t[:], scale=1.0
                )
                # switch ACT to the exp table set now (load hides under builds)
                nc.scalar.activation(out=tblw[:], in_=eps_t[:], func=fexp, scale=1.0)
                rstdg = small.tile([NG, 1], F32)
                nc.vector.reciprocal(out=rstdg[:], in_=stdg[:])
                gexp = small.tile([NG, 2], F32)
                nc.vector.tensor_copy(out=gexp[:, 0:1], in_=gstat[:, 0:1])
                nc.vector.tensor_copy(out=gexp[:, 1:2], in_=rstdg[:])
                mrc_ps = pp.tile([C, 2], F32, tag="pp")
                nc.tensor.matmul(mrc_ps[:], lhsT=c_g2[:], rhs=gexp[:])
                mrc = small.tile([C, 2], F32)
                nc.vector.tensor_copy(out=mrc[:], in_=mrc_ps[:])
                # scale_c = rstd_c * nw ; shift_c = nb - mean_c*scale_c
                scale_c = small.tile([C, 1], F32)
                nc.vector.tensor_tensor(
                    out=scale_c[:], in0=mrc[:, 1:2], in1=c_nw[:], op=MUL
                )
                shift_c = small.tile([C, 1], F32)
                nc.vector.tensor_tensor(
                    out=shift_c[:], in0=mrc[:, 0:1], in1=scale_c[:], op=MUL
                )
                nc.vector.tensor_tensor(
                    out=shift_c[:], in0=c_nb[:], in1=shift_c[:], op=SUB
                )
                # folded weights
                wq_f = consts.tile([C, 128], F32R)
                nc.vector.tensor_scalar_mul(out=wq_f[:], in0=c_wqT4[:], scalar1=scale_c[:])
                wk_f = consts.tile([C, 128], F32R)
                nc.vector.tensor_scalar_mul(out=wk_f[:], in0=c_wkT4[:], scalar1=scale_c[:])
                wv_f = consts.tile([C, DH], F32R)
                nc.vector.tensor_scalar_mul(out=wv_f[:], in0=c_wvT[:], scalar1=scale_c[:])
                # adjusted q bias (4x-replicated); k needs no bias at all
                bq_ps = pp.tile([128, 1], F32, tag="pp")
                nc.tensor.matmul(bq_ps[:], lhsT=c_wqT4[:], rhs=shift_c[:])
                bq_f = small.tile([128, 1], F32)
                nc.vector.tensor_tensor(
                    out=bq_f[:], in0=bq_ps[:], in1=c_bq4[:], op=ADD
                )
                # v shift term -> proj addvec rows (partitions 0 and 64)
                vs_ps = pp.tile([DH, 1], F32, tag="pp")
                nc.tensor.matmul(vs_ps[:], lhsT=c_wvT[:], rhs=shift_c[:])
                vsum = small.tile([DH, 1], F32)
                nc.vector.tensor_copy(out=vsum[:], in_=vs_ps[:])
                av_ps = pp.tile([97, C], F32, tag="av")
                nc.tensor.matmul(av_ps[32:33, :], lhsT=vsum[:], rhs=c_pwTa[:])
                nc.tensor.matmul(
                    av_ps[96:97, :], lhsT=vsum[:], rhs=c_pwTa[:],
                    tile_position=(0, 96),
                )
                nc.vector.tensor_copy(out=pwaug2[32:33, :], in_=av_ps[32:33, :])
                nc.vector.tensor_copy(out=pwaug2[96:97, :], in_=av_ps[96:97, :])
                # pwTa rows into both slabs (f32 -> f32r rounding producers)
                nc.vector.tensor_copy(out=pwaug2[0:32, :], in_=c_pwTa[:])
                nc.vector.tensor_copy(out=pwaug2[64:96, :], in_=c_pwTa[:])

                # ---------------- q/k builds ----------------
                for j in range(4):
                    o = 1024 * j
                    qp = buildp.tile([128, 1024], F32, tag="b")
                    nc.tensor.matmul(qp[:, 0:512], lhsT=wq_f[:], rhs=Xr[:, o : o + 512])
                    nc.tensor.matmul(
                        qp[:, 512:1024], lhsT=wq_f[:], rhs=Xr[:, o + 512 : o + 1024]
                    )
                    nc.scalar.activation(
                        out=q_sb[:, o : o + 1024], in_=qp[:], func=ident,
                        bias=bq_f[:], scale=1.0,
                    )
                    kp = buildp.tile([128, 1024], F32, tag="b")
                    nc.tensor.matmul(kp[:, 0:512], lhsT=wk_f[:], rhs=Xr[:, o : o + 512])
                    nc.tensor.matmul(
                        kp[:, 512:1024], lhsT=wk_f[:], rhs=Xr[:, o + 512 : o + 1024]
                    )
                    nc.vector.tensor_copy(out=k_sb[:, o : o + 1024], in_=kp[:])

            # ---------------- attention: 4 pairs of s-chunks ----------------
            # Groups of 2 units: 4 S-matmuls covering all 4 row bands run
            # concurrently (row tiling), then 2 AV pairs whose M=64 padded
            # lhsT lights the full array - keeps the HAM activity monitor
            # busy so the PE stays at 2.4 GHz.
            with (
                tc.tile_pool(name="sqp", bufs=3, space="PSUM") as sqp,
                tc.tile_pool(name="accp", bufs=1, space="PSUM") as accp,
            ):
                for p in range(4):
                    c0o = 1024 * p
                    c1o = 1024 * p + 512
                    acc = accp.tile([128, 512], F32, tag="acc")
                    acc1 = accp.tile([128, 512], F32, tag="acc1")
                    E_tiles = [None] * 32
                    for it in range(16 + GLAG):
                        g = it
                        if g < 16:
                            vp = None
                            if p == 0 and g < 8:
                                vp = sqp.tile([128, 128], F32, tag="sq")
                            for ui, u in enumerate((2 * g, 2 * g + 1)):
                                # K=128 over all 4 q/k replica bands computes
                                # 4*S in one full-array matmul: same 512-cycle
                                # stream as a K=32 matmul, but every PE cell
                                # carries real data, so the HAM activity
                                # monitor keeps the clock at 2.4 GHz.
                                SQ = sqp.tile([128, 1024], F32, tag="sq")
                                nc.tensor.matmul(
                                    SQ[:, 0:512],
                                    lhsT=k_sb[:, 128 * u : 128 * (u + 1)],
                                    rhs=q_sb[:, c0o : c0o + 512],
                                )
                                nc.tensor.matmul(
                                    SQ[:, 512:1024],
                                    lhsT=k_sb[:, 128 * u : 128 * (u + 1)],
                                    rhs=q_sb[:, c1o : c1o + 512],
                                )
                                if vp is not None:
                                    # v^T build rides along (blocks 4g..4g+3)
                                    for jj in range(2):
                                        tb = 4 * g + 2 * ui + jj
                                        nc.tensor.matmul(
                                            vp[:, 32 * (2 * ui + jj) : 32 * (2 * ui + jj + 1)],
                                            lhsT=Xr[:, 128 * tb : 128 * (tb + 1)],
                                            rhs=wv_f[:],
                                        )
                                if u in DVE_UNITS:
                                    Ei = epool.tile([128, 1024], I16, tag="ed", bufs=14)
                                    nc.vector.tensor_scalar(
                                        Ei[:], SQ[:], A_EXP, B_EXP, MUL, ADD
                                    )
                                    E_tiles[u] = Ei[:].bitcast(BF16)
                                else:
                                    Ea = epool.tile([128, 1024], BF16, tag="ea", bufs=18)
                                    nc.scalar.activation(
                                        out=Ea[:], in_=SQ[:], func=fexp, scale=float(SCALE / 4.0)
                                    )
                                    E_tiles[u] = Ea[:]
                            if vp is not None:
                                nc.vector.tensor_copy(
                                    out=v_t[:, 4 * g : 4 * (g + 1), 0:DH],
                                    in_=vp[:].rearrange("p (n d) -> p n d", d=32),
                                )
                        ga = it - GLAG
                        if 0 <= ga < 16:
                            for ua in (2 * ga, 2 * ga + 1):
                                E = E_tiles[ua]
                                E_tiles[ua] = None
                                # two accumulator slabs in SEPARATE banks (the
                                # start flag's has_written clear is bank-wide);
                                # col positions 0/64 + M=64 pad -> full-array
                                # concurrent accumulation streams
                                nc.tensor.matmul(
                                    acc[0:64, :],
                                    lhsT=v_t[:, ua, :],
                                    rhs=E[:, 0:512],
                                    tile_position=(0, 0),
                                    start=(ua == 0),
                                    stop=(ua == 31),
                                )
                                nc.tensor.matmul(
                                    acc1[64:128, :],
                                    lhsT=v_t[:, ua, :],
                                    rhs=E[:, 512:1024],
                                    tile_position=(0, 64),
                                    start=(ua == 0),
                                    stop=(ua == 31),
                                )

                    # ---------------- pair epilogue ----------------
                    U2 = epi.tile([97, 512], F32R, tag="u2")
                    nc.scalar.activation(
                        out=U2[0:33, :], in_=acc[0:33, :], func=fcopy, scale=1.0
                    )
                    nc.scalar.activation(
                        out=U2[64:97, :], in_=acc1[64:97, :], func=fcopy, scale=1.0
                    )
                    # denominators go to the host (proj is linear, so the
                    # host divides pj by denom during unsharding)
                    nc.sync.dma_start(
                        out=den_d[:, c0o : c0o + 512], in_=U2[32:33, :].bitcast(F32)
                    )
                    nc.sync.dma_start(
                        out=den_d[:, c1o : c1o + 512], in_=U2[96:97, :].bitcast(F32)
                    )
                    pj0 = accp.tile([C, 512], F32, tag="acc")
                    nc.tensor.matmul(
                        pj0[:], lhsT=pwaug2[0:33, :], rhs=U2[0:33, :]
                    )
                    osb0 = epi.tile([C, 512], F32, tag="o0")
                    nc.vector.tensor_copy(out=osb0[:], in_=pj0[:])
                    nc.sync.dma_start(out=out_d[:, c0o : c0o + 512], in_=osb0[:])
                    pj1 = accp.tile([C, 512], F32, tag="acc1")
                    nc.tensor.matmul(
                        pj1[:],
                        lhsT=pwaug2[64:97, :],
                        rhs=U2[64:97, :],
                        tile_position=(64, 0),
                    )
                    osb1 = epi.tile([C, 512], F32, tag="o1")
                    nc.vector.tensor_copy(out=osb1[:], in_=pj1[:])
                    nc.sync.dma_start(out=out_d[:, c1o : c1o + 512], in_=osb1[:])

    nc.compile()
    return nc


_NC_CACHE = None


def _get_program():
    global _NC_CACHE
    if _NC_CACHE is None:
        _NC_CACHE = build_program()
    return _NC_CACHE


def kernel(x, norm_w, norm_b, qkv_w, qkv_b, proj_w, proj_b):
    x = np.asarray(x, np.float32)
    norm_w = np.asarray(norm_w, np.float32)
    norm_b = np.asarray(norm_b, np.float32)
    qkv_w = np.asarray(qkv_w, np.float32)
    qkv_b = np.asarray(qkv_b, np.float32)
    proj_w = np.asarray(proj_w, np.float32)
    proj_b = np.asarray(proj_b, np.float32)

    nc = _get_program()

    gs = np.zeros((C, NG), np.float32)
    gs[np.arange(C), np.arange(C) // GS] = 1.0 / GS
    g2 = np.zeros((NG, C), np.float32)
    g2[np.arange(C) // GS, np.arange(C)] = 1.0

    in_maps = []
    for ci in range(N_CORES):
        b, h = ci // NH, ci % NH
        sl = slice(DH * h, DH * (h + 1))
        wqT = qkv_w[sl, :].T
        wkT = qkv_w[C:][sl, :].T
        cpk = np.concatenate(
            [
                np.tile(wqT, (1, 4)),
                np.tile(wkT, (1, 4)),
                qkv_w[2 * C :][sl, :].T,
                np.tile(qkv_b[sl].reshape(DH, 1), (4, 1)),
                gs,
                norm_w.reshape(C, 1),
                norm_b.reshape(C, 1),
            ],
            axis=1,
        )
        in_maps.append(
            {
                "xs": np.ascontiguousarray(x[b].reshape(C, HW)),
                "cpk": np.ascontiguousarray(cpk),
                "pwTa": np.ascontiguousarray(proj_w[:, sl].T),
                "g2": g2,
            }
        )

    res = run_bass_kernel_spmd(nc, in_maps, core_ids=list(range(N_CORES)))

    # unshard: sum per-head partials, add residual + proj bias + v-bias term
    base = proj_b + proj_w @ qkv_b[2 * C :]
    out = np.empty((B, C, HW), np.float32)
    for b in range(B):
        acc = np.zeros((C, HW), np.float32)
        for h in range(NH):
            r = res.results[b * NH + h]
            acc += r["out_p"] / r["den_p"]
        out[b] = acc + x[b].reshape(C, HW) + base[:, None]
    return out.reshape(B, C, 64, 64)
